# revision 6
# baseline (speedup 1.0000x reference)
"""Multi-head attention block (B=2, N=2048, C=1024, H=16, hd=64) on 8 TRN2 NeuronCores.

Sharding: data-parallel over batch (2 groups of 4 cores), tensor-parallel over
heads within each group (4 heads/core). Each core computes q/k/v for its heads,
attention, and a partial output projection; a ReduceScatter over the 4-core
group sums the partials, and the host reassembles the full [2, 2048, 1024]
output from the per-core shards.

Per-core layouts (everything transposed so the contraction dim sits on SBUF
partitions; the host pre-transposes x):
  xt   [1024, 2048]  x[b].T
  wqk  [1024, 512]   w_qkv columns for this core's q (256) ++ k (256)
  wv   [1024, 256]   w_qkv columns for this core's v
  wpb  [256, 1024]   w_proj rows for this core's heads
  bq   [128, 8]      b_proj/4, bq[p, m] = b_proj[m*128+p]/4
  out  [256, 2048]   rows g*256:(g+1)*256 of (x[b] @ ... ).T after RS
"""
import sys

if '/opt/trn_rl_repo' not in sys.path:
    sys.path.insert(0, '/opt/trn_rl_repo')

import numpy as np

import concourse.bass as bass
import concourse.mybir as mybir
import concourse.tile as tile
from concourse import bacc
from concourse.bass_utils import run_bass_kernel_spmd

F32 = mybir.dt.float32
F32R = mybir.dt.float32r

B = 2
N = 2048          # sequence length
C = 1024          # model dim
HEADS_PER_CORE = 4
HD = 64           # head dim
SCALE = HD ** -0.5
NT = N // 128     # 16 row tiles
CT = C // 128     # 8 contraction tiles
QC = 4            # q-chunks of 512
QCS = N // QC     # 512
GROUPS = [[0, 1, 2, 3], [4, 5, 6, 7]]

_NC_CACHE = None


def build():
    nc = bacc.Bacc(None, target_bir_lowering=False, debug=False)

    xt_ext = nc.declare_dram_parameter("xt", [C, N], F32, isOutput=False)
    wqk_ext = nc.declare_dram_parameter("wqk", [C, 512], F32, isOutput=False)
    wv_ext = nc.declare_dram_parameter("wv", [C, 256], F32, isOutput=False)
    wpb_ext = nc.declare_dram_parameter("wpb", [256, C], F32, isOutput=False)
    bq_ext = nc.declare_dram_parameter("bq", [128, 8], F32, isOutput=False)
    ones_ext = nc.declare_dram_parameter("ones64", [128, 64], F32, isOutput=False)
    out_ext = nc.declare_dram_parameter("out", [256, N], F32, isOutput=True)

    with tile.TileContext(nc) as tc:
        with (
            tc.tile_pool(name="weights", bufs=1) as wpool,
            tc.tile_pool(name="acts", bufs=1) as apool,
            tc.tile_pool(name="work", bufs=3) as work,
            tc.tile_pool(name="norm", bufs=2) as npool,
            tc.tile_pool(name="dram", bufs=2, space="DRAM") as dram,
        ):
            # ---- load inputs ----
            xt_sb = apool.tile([128, CT, N], F32R, tag="xt")
            wqk_sb = wpool.tile([128, CT, 512], F32R, tag="wqk")
            wv_sb = wpool.tile([128, CT, 256], F32R, tag="wv")
            wp_sb = wpool.tile([128, 2, C], F32R, tag="wp")
            bq_sb = wpool.tile([128, 8], F32, tag="bq")
            xt_r = xt_ext.ap().rearrange("(t p) n -> t p n", p=128)
            wqk_r = wqk_ext.ap().rearrange("(t p) n -> t p n", p=128)
            wv_r = wv_ext.ap().rearrange("(t p) n -> t p n", p=128)
            wpb_r = wpb_ext.ap().rearrange("(t p) n -> t p n", p=128)
            for ct in range(CT):
                nc.sync.dma_start(out=xt_sb[:, ct, :], in_=xt_r[ct].bitcast(F32R))
                nc.sync.dma_start(out=wqk_sb[:, ct, :], in_=wqk_r[ct].bitcast(F32R))
                nc.sync.dma_start(out=wv_sb[:, ct, :], in_=wv_r[ct].bitcast(F32R))
            for t in range(2):
                nc.sync.dma_start(out=wp_sb[:, t, :], in_=wpb_r[t].bitcast(F32R))
            nc.sync.dma_start(out=bq_sb[:, :], in_=bq_ext[:, :])

            # ---- phase A: qkT = wqk.T @ xt   [512, 2048], v = xt.T @ wv [2048, 256+ones] ----
            qk_sb = apool.tile([128, 4, N], F32R, tag="qk")
            v_sb = apool.tile([128, NT, HEADS_PER_CORE, HD + 1], F32R, tag="v")
            # ones column for the row-sum trick (memset can't write f32r tiles;
            # DMA from a host constant instead)
            nc.sync.dma_start(
                out=v_sb[:, :, :, HD:HD + 1],
                in_=ones_ext.ap().rearrange("p (a b c) -> p a b c", a=NT, b=HEADS_PER_CORE).bitcast(F32R),
            )
            with tc.tile_pool(name="psA", bufs=4, space="PSUM") as psA_pool:
                for m in range(4):
                    for qn in range(QC):
                        psA = psA_pool.tile([128, QCS], F32, tag="psA")
                        for ct in range(CT):
                            nc.tensor.matmul(
                                psA[:, :],
                                wqk_sb[:, ct, m * 128:(m + 1) * 128],
                                xt_sb[:, ct, qn * QCS:(qn + 1) * QCS],
                                start=(ct == 0), stop=(ct == CT - 1),
                            )
                        nc.vector.tensor_copy(qk_sb[:, m, qn * QCS:(qn + 1) * QCS], psA[:, :])
                for rt in range(NT):
                    psV = psA_pool.tile([128, 256], F32, tag="psV")
                    for ct in range(CT):
                        nc.tensor.matmul(
                            psV[:, :],
                            xt_sb[:, ct, rt * 128:(rt + 1) * 128],
                            wv_sb[:, ct, :],
                            start=(ct == 0), stop=(ct == CT - 1),
                        )
                    nc.vector.tensor_copy(
                        v_sb[:, rt, :, 0:HD],
                        psV[:, :].rearrange("p (h e) -> p h e", h=HEADS_PER_CORE),
                    )

            # ---- phases B/C/D per q-chunk ----
            with (
                tc.tile_pool(name="psS", bufs=2, space="PSUM") as psS_pool,
                tc.tile_pool(name="psO", bufs=1, space="PSUM") as psO_pool,
                tc.tile_pool(name="psP", bufs=2, space="PSUM") as psP_pool,
            ):
                for qc in range(QC):
                    qsl = slice(qc * QCS, (qc + 1) * QCS)
                    on_sb = npool.tile([128, 2, QCS], F32R, tag="on")
                    for pr in range(2):
                        psO_e = psO_pool.tile([65, QCS], F32, tag="psO_e")
                        psO_o = psO_pool.tile([65, QCS], F32, tag="psO_o")
                        for kt in range(NT):
                            ksl = slice(kt * 128, (kt + 1) * 128)
                            psS = psS_pool.tile([128, 2 * QCS], F32, tag="psS")
                            nc.tensor.matmul(
                                psS[:, 0:QCS],
                                qk_sb[0:64, 2 + pr, ksl],
                                qk_sb[0:64, pr, qsl],
                                start=True, stop=True,
                            )
                            nc.tensor.matmul(
                                psS[:, QCS:2 * QCS],
                                qk_sb[64:128, 2 + pr, ksl],
                                qk_sb[64:128, pr, qsl],
                                start=True, stop=True,
                            )
                            expt = work.tile([128, 2 * QCS], F32R, tag="expt")
                            nc.scalar.activation(
                                expt[:, :], psS[:, :],
                                mybir.ActivationFunctionType.Exp,
                                bias=0.0, scale=SCALE,
                            )
                            nc.tensor.matmul(
                                psO_e[:, :],
                                v_sb[:, kt, 2 * pr, :],
                                expt[:, 0:QCS],
                                start=(kt == 0), stop=(kt == NT - 1),
                            )
                            nc.tensor.matmul(
                                psO_o[:, :],
                                v_sb[:, kt, 2 * pr + 1, :],
                                expt[:, QCS:2 * QCS],
                                start=(kt == 0), stop=(kt == NT - 1),
                            )
                        # normalize: o / rowsum  (rowsum is psO[64], per q position)
                        for hh, psO in ((0, psO_e), (1, psO_o)):
                            recip = npool.tile([65, QCS], F32, tag="recip")
                            nc.vector.reciprocal(recip[64:65, :], psO[64:65, :])
                            row_dram = dram.tile([1, QCS], F32, tag="row")
                            nc.sync.dma_start(out=row_dram[:, :], in_=recip[64:65, :])
                            rd = row_dram[:, :]
                            bcast_src = bass.AP(
                                tensor=rd.tensor, offset=rd.offset,
                                ap=[[0, 64]] + list(rd.ap[1:]),
                            )
                            bcast = npool.tile([64, QCS], F32, tag="bcast")
                            nc.sync.dma_start(out=bcast[:, :], in_=bcast_src)
                            nc.vector.tensor_mul(
                                on_sb[hh * 64:(hh + 1) * 64, pr, :],
                                psO[0:64, :],
                                bcast[:, :],
                            )
                    # ---- projection + bias: partialT[m*128+p, q] ----
                    pT = dram.tile([C, QCS], F32, tag="pT")
                    for m in range(8):
                        psP = psP_pool.tile([128, QCS], F32, tag="psP")
                        for dst in range(2):
                            nc.tensor.matmul(
                                psP[:, :],
                                wp_sb[:, dst, m * 128:(m + 1) * 128],
                                on_sb[:, dst, :],
                                start=(dst == 0), stop=(dst == 1),
                            )
                        partial = work.tile([128, QCS], F32, tag="partial")
                        nc.vector.tensor_scalar_add(partial[:, :], psP[:, :], bq_sb[:, m:m + 1])
                        nc.sync.dma_start(out=pT[m * 128:(m + 1) * 128, :], in_=partial[:, :])
                    rs_out = dram.tile([256, QCS], F32, tag="rs")
                    nc.gpsimd.collective_compute(
                        "ReduceScatter",
                        mybir.AluOpType.add,
                        replica_groups=GROUPS,
                        ins=[pT.opt()],
                        outs=[rs_out.opt()],
                    )
                    nc.sync.dma_start(out=out_ext[:, qsl], in_=rs_out[:, :])

    nc.compile()
    return nc


def _get_nc():
    global _NC_CACHE
    if _NC_CACHE is None:
        _NC_CACHE = build()
    return _NC_CACHE


def shard_inputs(x, w_qkv, w_proj, b_proj):
    x = np.asarray(x, dtype=np.float32)
    w_qkv = np.asarray(w_qkv, dtype=np.float32)
    w_proj = np.asarray(w_proj, dtype=np.float32)
    b_proj = np.asarray(b_proj, dtype=np.float32)
    in_maps = []
    for core in range(8):
        b, g = divmod(core, 4)
        cs = slice(g * 256, (g + 1) * 256)
        wqk = np.concatenate([w_qkv[:, 0 * C + g * 256:0 * C + (g + 1) * 256],
                              w_qkv[:, 1 * C + g * 256:1 * C + (g + 1) * 256]], axis=1)
        in_maps.append({
            "xt": np.ascontiguousarray(x[b].T),
            "wqk": np.ascontiguousarray(wqk),
            "wv": np.ascontiguousarray(w_qkv[:, 2 * C + g * 256:2 * C + (g + 1) * 256]),
            "wpb": np.ascontiguousarray(w_proj[cs, :]),
            "bq": np.ascontiguousarray((b_proj / 4.0).reshape(8, 128).T),
            "ones64": np.ones((128, 64), dtype=np.float32),
        })
    return in_maps


def assemble_output(results):
    outT = np.empty((B, C, N), dtype=np.float32)
    for core in range(8):
        b, g = divmod(core, 4)
        outT[b, g * 256:(g + 1) * 256, :] = results[core]["out"]
    return np.ascontiguousarray(outT.transpose(0, 2, 1))


def run_sharded(x, w_qkv, w_proj, b_proj, trace=False):
    nc = _get_nc()
    in_maps = shard_inputs(x, w_qkv, w_proj, b_proj)
    res = run_bass_kernel_spmd(nc, in_maps, core_ids=list(range(8)), trace=trace)
    return assemble_output(res.results), res.exec_time_ns


def kernel(x, w_qkv, w_proj, b_proj):
    out, _ = run_sharded(x, w_qkv, w_proj, b_proj, trace=False)
    return out
